# revision 14
# baseline (speedup 1.0000x reference)
"""Sliding-window causal attention (B=2, H=16, T=2048, D=64, WINDOW=512) on
8 TRN2 NeuronCores.

Sharding: the 32 (b, h) pairs are split 4-per-core (embarrassingly parallel).
Each core runs the same Bass/Tile program over its 4 heads (2 pairs).

v2 structure (vs baseline):
  - QK matmuls of the two heads of a pair are interleaved chunk-by-chunk in
    PE program order; head A lives on partitions 0:64, head B on 64:128, so
    their K=64 matmuls auto-derive tile_position (0,0)/(64,0) and run
    CONCURRENTLY in different row-groups of the systolic array (~2x QK).
  - PV accumulates straight into a full-bank PSUM tile per query block; the
    softmax normalize (reciprocal + broadcast mul) reads PSUM directly and
    writes the staged output tile -- the 64 per-block drain copies are gone.
  - Masking is a single multiplicative bf16 tensor_mul per boundary subtile
    on gpsimd (const 0/1 triangle masks) instead of affine_selects.
  - Staging DMAs fetch both heads of a pair in one dma_start; V is fetched
    with one whole-head DMA.  Pair 1's staging is fed into pair 0's
    attention loop so PE/ACT never drain at the pair boundary.
  - exp runs on ACT (the steady-state bottleneck: ~9us/head); everything
    movable (casts, drains, normalize muls) uses nc.any so the Tile
    scheduler balances DVE/Pool.
"""

import sys
from contextlib import ExitStack

import numpy as np

sys.path.insert(0, "/opt/trn_rl_repo")

import concourse.bacc as bacc
import concourse.tile as tile
from concourse import mybir
from concourse.bass_utils import run_bass_kernel_spmd

F32 = mybir.dt.float32
BF16 = mybir.dt.bfloat16
EXP = mybir.ActivationFunctionType.Exp

B, H, T, D = 2, 16, 2048, 64
WINDOW = 512
SCALE = D ** -0.5
N_CORES = 8
HEADS_PER_CORE = (B * H) // N_CORES  # 4
TB = T // 128  # 16 query/key blocks
TH = 1024  # half-sequence tile width for qd/kd


def build_nc(t=T, heads_per_core=HEADS_PER_CORE):
    nb = t // 128

    nc = bacc.Bacc("TRN2", target_bir_lowering=False)
    q_ext = nc.declare_dram_parameter("q", [heads_per_core, t, D], F32, isOutput=False)
    k_ext = nc.declare_dram_parameter("k", [heads_per_core, t, D], F32, isOutput=False)
    v_ext = nc.declare_dram_parameter("v", [heads_per_core, t, D], F32, isOutput=False)
    id_ext = nc.declare_dram_parameter("ident", [128, 128], F32, isOutput=False)
    o_ext = nc.declare_dram_parameter("out", [heads_per_core, t, D], F32, isOutput=True)

    assert heads_per_core % 2 == 0
    n_pairs = heads_per_core // 2

    with tile.TileContext(nc) as tc, ExitStack() as ctx:
        const = ctx.enter_context(tc.tile_pool(name="const", bufs=1))
        stage = ctx.enter_context(tc.tile_pool(name="stage", bufs=4))
        vstage = ctx.enter_context(tc.tile_pool(name="vstage", bufs=2))
        qkd = ctx.enter_context(tc.tile_pool(name="qkd", bufs=2))
        vps = ctx.enter_context(tc.tile_pool(name="vps", bufs=4))
        ets = ctx.enter_context(tc.tile_pool(name="ets", bufs=13))
        outs = ctx.enter_context(tc.tile_pool(name="outs", bufs=3))
        rcp = ctx.enter_context(tc.tile_pool(name="rcp", bufs=4))
        # PSUM: 1 (trp) + 2*2 (sp) + 2*1 (ob) + 1 (dummy) = 8 banks exactly
        tr_ps = ctx.enter_context(tc.tile_pool(name="tr_ps", bufs=1, space="PSUM"))
        s_ps = ctx.enter_context(tc.tile_pool(name="s_ps", bufs=2, space="PSUM"))
        ob_ps = ctx.enter_context(tc.tile_pool(name="ob_ps", bufs=2, space="PSUM"))
        dm_ps = ctx.enter_context(tc.tile_pool(name="dm_ps", bufs=1, space="PSUM"))

        # HAM warmup: the PE clock gate only opens (1.2 -> 2.4 GHz) after
        # ~3.4us of sustained matmul activity, and every real matmul here
        # runs ~2x faster warm.  Burn a dense burst of dummy matmuls on a
        # scratch PSUM bank while the first DMAs are in flight, and keep
        # feeding short dummy bursts between real matmul groups so the
        # activity monitor never re-throttles.
        dm_src = const.tile([128, 128], BF16, tag="dm_src")
        nc.vector.memset(dm_src[:], 0.0)
        dm_out = dm_ps.tile([128, 512], F32, tag="dm_out")

        def pe_dummy(n):
            for i in range(n):
                nc.tensor.matmul(
                    dm_out[:, 0:128], dm_src[:], dm_src[:], start=True, stop=True
                )

        pe_dummy(30)

        # fp32 identity + bf16 copy (for Q/K transposes).
        ident_f = const.tile([128, 128], F32, tag="ident_f")
        nc.sync.dma_start(out=ident_f[:], in_=id_ext[:])
        ident_b = const.tile([128, 128], BF16, tag="ident_b")
        nc.vector.tensor_copy(ident_b[:], ident_f[:])

        # multiplicative 0/1 masks for the two boundary subtiles of E^T,
        # packed [128, 2, 128] so one strided tensor_mul masks both:
        # slot 0 keeps c >= r (causal diagonal), slot 1 keeps c < r (window).
        maskDW = const.tile([128, 2, 128], BF16, tag="maskDW")
        nc.gpsimd.memset(maskDW[:], 1.0)
        nc.gpsimd.affine_select(
            out=maskDW[:, 0, :], in_=maskDW[:, 0, :],
            compare_op=mybir.AluOpType.is_ge,
            fill=0.0, base=0, pattern=[[1, 128]], channel_multiplier=-1,
        )
        nc.gpsimd.affine_select(
            out=maskDW[:, 1, :], in_=maskDW[:, 1, :],
            compare_op=mybir.AluOpType.is_ge,
            fill=0.0, base=-1, pattern=[[-1, 128]], channel_multiplier=1,
        )

        # per-pair state
        qd_halves = {}
        kd_halves = {}
        vp = {}

        def alloc_pair(pair):
            qd_halves[pair] = [
                qkd.tile([128, TH], BF16, tag="qd0", name=f"qd0_{pair}"),
                qkd.tile([128, TH], BF16, tag="qd1", name=f"qd1_{pair}"),
            ]
            kd_halves[pair] = [
                qkd.tile([128, TH], BF16, tag="kd0", name=f"kd0_{pair}"),
                qkd.tile([128, TH], BF16, tag="kd1", name=f"kd1_{pair}"),
            ]

        def stage_unit(pair, ext, halves, u):
            # one 512-row chunk of q or k, both heads, DMA -> cast ->
            # 4 PE transposes -> drain into the d-major [128, TH] half.
            rows = slice(u * 512, (u + 1) * 512)
            st_f = stage.tile([128, 512], F32, tag="st_f")
            st3 = st_f[:].rearrange("p (b c) -> p b c", c=128)
            for hh, doff in ((2 * pair, 0), (2 * pair + 1, 64)):
                nc.sync.dma_start(
                    out=st3[:, :, doff : doff + 64],
                    in_=ext[hh, rows, :].rearrange("(b p) d -> p b d", p=128),
                )
            st_b = stage.tile([128, 512], BF16, tag="st_b")
            nc.vector.tensor_copy(st_b[:], st_f[:])
            trp = tr_ps.tile([128, 1024], BF16, tag="trp")
            for i in range(4):
                nc.tensor.transpose(
                    trp[:, i * 128 : (i + 1) * 128],
                    st_b[:, i * 128 : (i + 1) * 128],
                    ident_b[:],
                )
            dst = halves[u // 2]
            dcol = (u % 2) * 512
            nc.vector.tensor_copy(dst[:, dcol : dcol + 512], trp[:, 0:512])

        def stage_v(h):
            vst = vstage.tile([128, 1024], F32, tag="vst")
            v3 = vst[:].rearrange("p (b d) -> p b d", d=64)
            nc.sync.dma_start(
                out=v3, in_=v_ext[h].rearrange("(b p) d -> p b d", p=128)
            )
            vt = vps.tile([128, nb, 65], BF16, tag="vp", name=f"vp_{h}")
            nc.gpsimd.tensor_copy(vt[:, :, 0:64], v3)
            nc.gpsimd.memset(vt[:, :, 64:65], 1.0)
            vp[h] = vt

        def stage_feed(pair):
            # closures that stage pair `pair`, to be interleaved into the
            # previous pair's attention loop (or run immediately).
            alloc_pair(pair)
            units = []
            units.append(lambda: stage_unit(pair, q_ext, qd_halves[pair], 0))
            units.append(lambda: stage_unit(pair, k_ext, kd_halves[pair], 0))
            units.append(lambda: stage_v(2 * pair))
            units.append(lambda: stage_v(2 * pair + 1))
            units.append(lambda: stage_unit(pair, q_ext, qd_halves[pair], 1))
            units.append(lambda: stage_unit(pair, k_ext, kd_halves[pair], 1))
            for u in (2, 3):
                units.append(lambda u=u: stage_unit(pair, q_ext, qd_halves[pair], u))
                units.append(lambda u=u: stage_unit(pair, k_ext, kd_halves[pair], u))
            return units

        def attention(pair, feed):
            # feed: dict kb -> list of closures (next pair's staging)
            hA, hB = 2 * pair, 2 * pair + 1
            rows_of = {hA: slice(0, 64), hB: slice(64, 128)}
            qdh, kdh = qd_halves[pair], kd_halves[pair]
            et = {hA: {}, hB: {}}
            sp_t = {}
            oo_t = {}
            oo3 = {}

            def emit_qk(kb):
                a = kb * 128
                span = min(640, t - a)
                for h in (hA, hB):
                    sp_t[h] = s_ps.tile([128, 1024], F32, tag="sp", name=f"sp_{h}_{kb}")
                chunks = []
                for lo in (0, TH):
                    s0, s1 = max(a, lo), min(a + span, lo + TH)
                    while s0 < s1:
                        n = min(512 - (s0 - a) % 512, s1 - s0)
                        chunks.append((s0 - a, lo // TH, s0 - lo, n))
                        s0 += n
                kd_half = kdh[a // TH]
                kcol = a % TH
                for (c, half, qc, n) in chunks:
                    for h in (hA, hB):
                        r = rows_of[h]
                        nc.tensor.matmul(
                            sp_t[h][:, c : c + n],
                            kd_half[r, kcol : kcol + 128],
                            qdh[half][r, qc : qc + n],
                            start=True,
                            stop=True,
                        )
                for h in (hA, hB):
                    e = ets.tile([128, 640], BF16, tag="et", name=f"et_{h}_{kb}")
                    et[h][kb] = e
                    nc.scalar.activation(e[:, 0:span], sp_t[h][:, 0:span], EXP, scale=SCALE)
                    if span == 640:
                        # one strided op masks both boundary triangles
                        e3 = e[:].rearrange("p (a b) -> p a b", b=128)
                        nc.vector.tensor_mul(
                            e3[:, 0:5:4, :], e3[:, 0:5:4, :], maskDW[:]
                        )
                    else:
                        nc.vector.tensor_mul(
                            e[:, 0:128], e[:, 0:128], maskDW[:, 0, :]
                        )

            ob_t = {}

            def emit_pv(qb):
                g, j = qb // 4, qb % 4
                for h in (hA, hB):
                    if j == 0:
                        oo_t[h] = outs.tile([128, 256], F32, tag="oo", name=f"oo_{h}_{g}")
                        oo3[h] = oo_t[h][:].rearrange("p (b d) -> p b d", d=64)
                        # 4 query blocks accumulate into one PSUM bank
                        ob_t[h] = ob_ps.tile([128, 512], F32, tag="ob", name=f"ob_{h}_{g}")
                    ob = ob_t[h]
                    kb0 = max(0, qb - 4)
                    for kb in range(kb0, qb + 1):
                        nc.tensor.matmul(
                            ob[:, j * 65 : j * 65 + 65],
                            et[h][kb][:, (qb - kb) * 128 : (qb - kb) * 128 + 128],
                            vp[h][:, kb, :],
                            start=(kb == kb0),
                            stop=(kb == qb),
                        )
                    if qb >= 4:
                        del et[h][qb - 4]
                    if j == 3 or qb == nb - 1:
                        # batched normalize straight out of PSUM
                        nj = j + 1
                        ob3 = ob[:, 0 : nj * 65].rearrange("p (b c) -> p b c", c=65)
                        rc = rcp.tile([128, 4], F32, tag="rc")
                        nc.vector.reciprocal(rc[:, 0:nj], ob3[:, 0:nj, 64])
                        nc.vector.tensor_mul(
                            oo3[h][:, 0:nj, :],
                            ob3[:, 0:nj, 0:64],
                            rc[:, 0:nj].rearrange("p (b c) -> p b c", c=1).broadcast_to(
                                [128, nj, 64]
                            ),
                        )
                        nc.sync.dma_start(
                            out=o_ext[h, g * 512 : g * 512 + nj * 128, :].rearrange(
                                "(b p) d -> p b d", p=128
                            ),
                            in_=oo3[h][:, 0:nj, :],
                        )

            for kb in range(nb + 1):
                if kb < nb:
                    emit_qk(kb)
                if kb >= 1:
                    emit_pv(kb - 1)
                    # keep the PE activity monitor warm: dummy matmuls
                    # tethered to the freshest E tile so the scheduler
                    # spreads them through the timeline instead of
                    # hoisting them to t=0
                    for h in (hA, hB):
                        if kb - 1 in et[h]:
                            for _ in range(2):
                                nc.tensor.matmul(
                                    dm_out[:, 0:128],
                                    et[h][kb - 1][:, 0:128],
                                    dm_src[:],
                                    start=True,
                                    stop=True,
                                )
                for fn in feed.get(kb, ()):
                    fn()

        # pair 0: stage the first chunks inline, rest fed into its own loop
        units0 = stage_feed(0)
        for fn in units0[:6]:
            fn()
        feed0 = {0: [units0[6], units0[7]], 1: [units0[8], units0[9]]}
        # pair 1 staged during pair 0's attention, starting at kb=6
        units1 = stage_feed(1)
        feed1_in_0 = {6 + i: [units1[i]] for i in range(len(units1))}
        feed0.update(feed1_in_0)

        attention(0, feed0)
        attention(1, {})

        # drain the dummy bank so every written tile has a reader
        dm_sink = const.tile([128, 1], F32, tag="dm_sink")
        nc.vector.tensor_copy(dm_sink[:], dm_out[:, 0:1])

    nc.compile()
    return nc


_NC_CACHE = {}
TRACE = False
TRACE_DIR = None
LAST_RESULT = None


def _get_nc():
    key = (T, HEADS_PER_CORE)
    if key not in _NC_CACHE:
        _NC_CACHE[key] = build_nc()
    return _NC_CACHE[key]


def kernel(q, k, v):
    q = np.ascontiguousarray(np.asarray(q, dtype=np.float32))
    k = np.ascontiguousarray(np.asarray(k, dtype=np.float32))
    v = np.ascontiguousarray(np.asarray(v, dtype=np.float32))
    assert q.shape == (B, H, T, D)

    qf = q.reshape(B * H, T, D)
    kf = k.reshape(B * H, T, D)
    vf = v.reshape(B * H, T, D)
    ident = np.eye(128, dtype=np.float32)

    in_maps = []
    for c in range(N_CORES):
        s = slice(c * HEADS_PER_CORE, (c + 1) * HEADS_PER_CORE)
        in_maps.append(
            {
                "q": np.ascontiguousarray(qf[s]),
                "k": np.ascontiguousarray(kf[s]),
                "v": np.ascontiguousarray(vf[s]),
                "ident": ident,
            }
        )

    nc = _get_nc()
    global LAST_RESULT
    res = run_bass_kernel_spmd(
        nc, in_maps, list(range(N_CORES)), trace=TRACE, tmpdir=TRACE_DIR
    )
    LAST_RESULT = res
    out = np.concatenate([res.results[c]["out"] for c in range(N_CORES)], axis=0)
    return out.reshape(B, H, T, D).astype(np.float32)


# revision 16
# speedup vs baseline: 1.2297x; 1.2297x over previous
"""Sliding-window causal attention (B=2, H=16, T=2048, D=64, WINDOW=512) on
8 TRN2 NeuronCores.

Sharding: the 32 (b, h) pairs are split 4-per-core (embarrassingly parallel).
Each core runs the same Bass/Tile program over its 4 heads (2 pairs).

v2 structure (vs baseline):
  - QK matmuls of the two heads of a pair are interleaved chunk-by-chunk in
    PE program order; head A lives on partitions 0:64, head B on 64:128, so
    their K=64 matmuls auto-derive tile_position (0,0)/(64,0) and run
    CONCURRENTLY in different row-groups of the systolic array (~2x QK).
  - PV accumulates straight into a full-bank PSUM tile per query block; the
    softmax normalize (reciprocal + broadcast mul) reads PSUM directly and
    writes the staged output tile -- the 64 per-block drain copies are gone.
  - Masking is a single multiplicative bf16 tensor_mul per boundary subtile
    on gpsimd (const 0/1 triangle masks) instead of affine_selects.
  - Staging DMAs fetch both heads of a pair in one dma_start; V is fetched
    with one whole-head DMA.  Pair 1's staging is fed into pair 0's
    attention loop so PE/ACT never drain at the pair boundary.
  - exp runs on ACT (the steady-state bottleneck: ~9us/head); everything
    movable (casts, drains, normalize muls) uses nc.any so the Tile
    scheduler balances DVE/Pool.
"""

import sys
from contextlib import ExitStack

import numpy as np

sys.path.insert(0, "/opt/trn_rl_repo")

import concourse.bacc as bacc
import concourse.tile as tile
from concourse import mybir
from concourse.bass_utils import run_bass_kernel_spmd

F32 = mybir.dt.float32
BF16 = mybir.dt.bfloat16
EXP = mybir.ActivationFunctionType.Exp

B, H, T, D = 2, 16, 2048, 64
WINDOW = 512
SCALE = D ** -0.5
N_CORES = 8
HEADS_PER_CORE = (B * H) // N_CORES  # 4
TB = T // 128  # 16 query/key blocks
TH = 1024  # half-sequence tile width for qd/kd


def build_nc(t=T, heads_per_core=HEADS_PER_CORE):
    nb = t // 128

    nc = bacc.Bacc("TRN2", target_bir_lowering=False)
    q_ext = nc.declare_dram_parameter("q", [heads_per_core, t, D], F32, isOutput=False)
    k_ext = nc.declare_dram_parameter("k", [heads_per_core, t, D], F32, isOutput=False)
    v_ext = nc.declare_dram_parameter("v", [heads_per_core, t, D], F32, isOutput=False)
    id_ext = nc.declare_dram_parameter("ident", [128, 128], F32, isOutput=False)
    o_ext = nc.declare_dram_parameter("out", [heads_per_core, t, D], F32, isOutput=True)

    assert heads_per_core % 2 == 0
    n_pairs = heads_per_core // 2

    with tile.TileContext(nc) as tc, ExitStack() as ctx:
        const = ctx.enter_context(tc.tile_pool(name="const", bufs=1))
        stage = ctx.enter_context(tc.tile_pool(name="stage", bufs=4))
        vstage = ctx.enter_context(tc.tile_pool(name="vstage", bufs=2))
        qkd = ctx.enter_context(tc.tile_pool(name="qkd", bufs=2))
        vps = ctx.enter_context(tc.tile_pool(name="vps", bufs=4))
        ets = ctx.enter_context(tc.tile_pool(name="ets", bufs=13))
        outs = ctx.enter_context(tc.tile_pool(name="outs", bufs=3))
        rcp = ctx.enter_context(tc.tile_pool(name="rcp", bufs=4))
        # PSUM banks: 1 (trp) + 3*2 (sp) + 1 (shared ob/warmup) = 8
        tr_ps = ctx.enter_context(tc.tile_pool(name="tr_ps", bufs=1, space="PSUM"))
        s_ps = ctx.enter_context(tc.tile_pool(name="s_ps", bufs=3, space="PSUM"))
        ob_ps = ctx.enter_context(tc.tile_pool(name="ob_ps", bufs=1, space="PSUM"))

        # HAM warmup: the PE clock gate only opens (1.2 -> 2.4 GHz) after
        # ~3.4us of sustained matmul activity, and every real matmul here
        # runs ~2x faster warm.  Burn a dense burst of dummy matmuls on a
        # scratch PSUM bank while the first DMAs are in flight, and keep
        # feeding short dummy bursts between real matmul groups so the
        # activity monitor never re-throttles.
        dm_src = const.tile([128, 128], BF16, tag="dm_src")
        nc.vector.memset(dm_src[:], 0.0)
        dm_out = ob_ps.tile([128, 512], F32, tag="ob", name="ob_warm")

        def pe_dummy(n):
            for i in range(n):
                nc.tensor.matmul(
                    dm_out[:, 384:512], dm_src[:], dm_src[:], start=True, stop=True
                )

        pe_dummy(30)
        dm_sink = const.tile([128, 1], F32, tag="dm_sink")
        nc.vector.tensor_copy(dm_sink[:], dm_out[:, 384:385])

        # fp32 identity + bf16 copy (for Q/K transposes).
        ident_f = const.tile([128, 128], F32, tag="ident_f")
        nc.sync.dma_start(out=ident_f[:], in_=id_ext[:])
        ident_b = const.tile([128, 128], BF16, tag="ident_b")
        nc.vector.tensor_copy(ident_b[:], ident_f[:])

        # multiplicative 0/1 masks for the two boundary subtiles of E^T,
        # packed [128, 2, 128] so one strided tensor_mul masks both:
        # slot 0 keeps c >= r (causal diagonal), slot 1 keeps c < r (window).
        maskDW = const.tile([128, 2, 128], BF16, tag="maskDW")
        nc.gpsimd.memset(maskDW[:], 1.0)
        nc.gpsimd.affine_select(
            out=maskDW[:, 0, :], in_=maskDW[:, 0, :],
            compare_op=mybir.AluOpType.is_ge,
            fill=0.0, base=0, pattern=[[1, 128]], channel_multiplier=-1,
        )
        nc.gpsimd.affine_select(
            out=maskDW[:, 1, :], in_=maskDW[:, 1, :],
            compare_op=mybir.AluOpType.is_ge,
            fill=0.0, base=-1, pattern=[[-1, 128]], channel_multiplier=1,
        )

        # per-pair state
        qd_halves = {}
        kd_halves = {}
        vp = {}

        def alloc_pair(pair):
            qd_halves[pair] = [
                qkd.tile([128, TH], BF16, tag="qd0", name=f"qd0_{pair}"),
                qkd.tile([128, TH], BF16, tag="qd1", name=f"qd1_{pair}"),
            ]
            kd_halves[pair] = [
                qkd.tile([128, TH], BF16, tag="kd0", name=f"kd0_{pair}"),
                qkd.tile([128, TH], BF16, tag="kd1", name=f"kd1_{pair}"),
            ]

        def stage_unit(pair, ext, halves, u):
            # one 512-row chunk of q or k, both heads, DMA -> cast ->
            # 4 PE transposes -> drain into the d-major [128, TH] half.
            rows = slice(u * 512, (u + 1) * 512)
            st_f = stage.tile([128, 512], F32, tag="st_f")
            st3 = st_f[:].rearrange("p (b c) -> p b c", c=128)
            for hh, doff in ((2 * pair, 0), (2 * pair + 1, 64)):
                nc.sync.dma_start(
                    out=st3[:, :, doff : doff + 64],
                    in_=ext[hh, rows, :].rearrange("(b p) d -> p b d", p=128),
                )
            st_b = stage.tile([128, 512], BF16, tag="st_b")
            nc.vector.tensor_copy(st_b[:], st_f[:])
            trp = tr_ps.tile([128, 512], BF16, tag="trp")
            for i in range(4):
                nc.tensor.transpose(
                    trp[:, i * 128 : (i + 1) * 128],
                    st_b[:, i * 128 : (i + 1) * 128],
                    ident_b[:],
                )
            dst = halves[u // 2]
            dcol = (u % 2) * 512
            nc.vector.tensor_copy(dst[:, dcol : dcol + 512], trp[:, 0:512])

        def stage_v(h):
            vst = vstage.tile([128, 1024], F32, tag="vst")
            v3 = vst[:].rearrange("p (b d) -> p b d", d=64)
            nc.sync.dma_start(
                out=v3, in_=v_ext[h].rearrange("(b p) d -> p b d", p=128)
            )
            vt = vps.tile([128, nb, 65], BF16, tag="vp", name=f"vp_{h}")
            nc.gpsimd.tensor_copy(vt[:, :, 0:64], v3)
            nc.gpsimd.memset(vt[:, :, 64:65], 1.0)
            vp[h] = vt

        def stage_feed(pair):
            # closures that stage pair `pair`, to be interleaved into the
            # previous pair's attention loop (or run immediately).
            alloc_pair(pair)
            units = []
            units.append(lambda: stage_unit(pair, q_ext, qd_halves[pair], 0))
            units.append(lambda: stage_unit(pair, k_ext, kd_halves[pair], 0))
            units.append(lambda: stage_v(2 * pair))
            units.append(lambda: stage_v(2 * pair + 1))
            units.append(lambda: stage_unit(pair, q_ext, qd_halves[pair], 1))
            units.append(lambda: stage_unit(pair, k_ext, kd_halves[pair], 1))
            for u in (2, 3):
                units.append(lambda u=u: stage_unit(pair, q_ext, qd_halves[pair], u))
                units.append(lambda u=u: stage_unit(pair, k_ext, kd_halves[pair], u))
            return units

        def attention(pair, feed):
            # feed: dict kb -> list of closures (next pair's staging)
            hA, hB = 2 * pair, 2 * pair + 1
            rows_of = {hA: slice(0, 64), hB: slice(64, 128)}
            qdh, kdh = qd_halves[pair], kd_halves[pair]
            et = {hA: {}, hB: {}}
            sp_t = {}
            oo_t = {}
            oo3 = {}

            def emit_qk(kb):
                a = kb * 128
                span = min(640, t - a)
                for h in (hA, hB):
                    sp_t[h] = s_ps.tile([128, 1024], F32, tag="sp", name=f"sp_{h}_{kb}")
                chunks = []
                for lo in (0, TH):
                    s0, s1 = max(a, lo), min(a + span, lo + TH)
                    while s0 < s1:
                        n = min(512 - (s0 - a) % 512, s1 - s0)
                        chunks.append((s0 - a, lo // TH, s0 - lo, n))
                        s0 += n
                kd_half = kdh[a // TH]
                kcol = a % TH
                for (c, half, qc, n) in chunks:
                    for h in (hA, hB):
                        r = rows_of[h]
                        nc.tensor.matmul(
                            sp_t[h][:, c : c + n],
                            kd_half[r, kcol : kcol + 128],
                            qdh[half][r, qc : qc + n],
                            start=True,
                            stop=True,
                        )
                for h in (hA, hB):
                    e = ets.tile([128, 640], BF16, tag="et", name=f"et_{h}_{kb}")
                    et[h][kb] = e
                    nc.scalar.activation(e[:, 0:span], sp_t[h][:, 0:span], EXP, scale=SCALE)
                    if span == 640:
                        # one strided op masks both boundary triangles
                        e3 = e[:].rearrange("p (a b) -> p a b", b=128)
                        nc.vector.tensor_mul(
                            e3[:, 0:5:4, :], e3[:, 0:5:4, :], maskDW[:]
                        )
                    else:
                        nc.vector.tensor_mul(
                            e[:, 0:128], e[:, 0:128], maskDW[:, 0, :]
                        )

            ob_t = {}

            def emit_pv(qb):
                g, j = qb // 4, qb % 4
                g2, j2 = qb // 2, qb % 2
                for h in (hA, hB):
                    if j == 0:
                        oo_t[h] = outs.tile([128, 256], F32, tag="oo", name=f"oo_{h}_{g}")
                        oo3[h] = oo_t[h][:].rearrange("p (b d) -> p b d", d=64)
                    if j2 == 0 and h == hA:
                        # both heads' 2-qb accumulators share one bank-tile
                        ob_t[0] = ob_ps.tile([128, 512], F32, tag="ob", name=f"ob_{pair}_{g2}")
                    hoff = 0 if h == hA else 130
                    ob = ob_t[0][:, hoff : hoff + 130].rearrange(
                        "p (b c) -> p b c", c=65
                    )
                    kb0 = max(0, qb - 4)
                    for kb in range(kb0, qb + 1):
                        nc.tensor.matmul(
                            ob[:, j2, :],
                            et[h][kb][:, (qb - kb) * 128 : (qb - kb) * 128 + 128],
                            vp[h][:, kb, :],
                            start=(kb == kb0),
                            stop=(kb == qb),
                        )
                    if qb >= 4:
                        del et[h][qb - 4]
                    if j2 == 1:
                        # normalize 2 query blocks straight out of PSUM
                        rc = rcp.tile([128, 2], F32, tag="rc")
                        nc.vector.reciprocal(rc[:], ob[:, :, 64])
                        nc.vector.tensor_mul(
                            oo3[h][:, j - 1 : j + 1, :],
                            ob[:, :, 0:64],
                            rc[:].rearrange("p (b c) -> p b c", c=1).broadcast_to(
                                [128, 2, 64]
                            ),
                        )
                    if j == 3 or qb == nb - 1:
                        nc.sync.dma_start(
                            out=o_ext[h, g * 512 : g * 512 + 512, :].rearrange(
                                "(b p) d -> p b d", p=128
                            ),
                            in_=oo3[h][:, 0:4, :],
                        )

            for kb in range(nb + 1):
                if kb < nb:
                    emit_qk(kb)
                if kb >= 1:
                    emit_pv(kb - 1)
                for fn in feed.get(kb, ()):
                    fn()

        # pair 0: stage the first chunks inline, rest fed into its own loop
        units0 = stage_feed(0)
        for fn in units0[:6]:
            fn()
        feed0 = {0: [units0[6], units0[7]], 1: [units0[8], units0[9]]}
        # pair 1 staged during pair 0's attention, starting at kb=6
        units1 = stage_feed(1)
        feed1_in_0 = {6 + i: [units1[i]] for i in range(len(units1))}
        feed0.update(feed1_in_0)

        attention(0, feed0)
        attention(1, {})


    nc.compile()
    return nc


_NC_CACHE = {}
TRACE = False
TRACE_DIR = None
LAST_RESULT = None


def _get_nc():
    key = (T, HEADS_PER_CORE)
    if key not in _NC_CACHE:
        _NC_CACHE[key] = build_nc()
    return _NC_CACHE[key]


def kernel(q, k, v):
    q = np.ascontiguousarray(np.asarray(q, dtype=np.float32))
    k = np.ascontiguousarray(np.asarray(k, dtype=np.float32))
    v = np.ascontiguousarray(np.asarray(v, dtype=np.float32))
    assert q.shape == (B, H, T, D)

    qf = q.reshape(B * H, T, D)
    kf = k.reshape(B * H, T, D)
    vf = v.reshape(B * H, T, D)
    ident = np.eye(128, dtype=np.float32)

    in_maps = []
    for c in range(N_CORES):
        s = slice(c * HEADS_PER_CORE, (c + 1) * HEADS_PER_CORE)
        in_maps.append(
            {
                "q": np.ascontiguousarray(qf[s]),
                "k": np.ascontiguousarray(kf[s]),
                "v": np.ascontiguousarray(vf[s]),
                "ident": ident,
            }
        )

    nc = _get_nc()
    global LAST_RESULT
    res = run_bass_kernel_spmd(
        nc, in_maps, list(range(N_CORES)), trace=TRACE, tmpdir=TRACE_DIR
    )
    LAST_RESULT = res
    out = np.concatenate([res.results[c]["out"] for c in range(N_CORES)], axis=0)
    return out.reshape(B, H, T, D).astype(np.float32)


# revision 17
# speedup vs baseline: 1.2711x; 1.0337x over previous
"""Sliding-window causal attention (B=2, H=16, T=2048, D=64, WINDOW=512) on
8 TRN2 NeuronCores.

Sharding: the 32 (b, h) pairs are split 4-per-core (embarrassingly parallel).
Each core runs the same Bass/Tile program over its 4 heads (2 pairs).

v2 structure (vs baseline):
  - QK matmuls of the two heads of a pair are interleaved chunk-by-chunk in
    PE program order; head A lives on partitions 0:64, head B on 64:128, so
    their K=64 matmuls auto-derive tile_position (0,0)/(64,0) and run
    CONCURRENTLY in different row-groups of the systolic array (~2x QK).
  - PV accumulates straight into a full-bank PSUM tile per query block; the
    softmax normalize (reciprocal + broadcast mul) reads PSUM directly and
    writes the staged output tile -- the 64 per-block drain copies are gone.
  - Masking is a single multiplicative bf16 tensor_mul per boundary subtile
    on gpsimd (const 0/1 triangle masks) instead of affine_selects.
  - Staging DMAs fetch both heads of a pair in one dma_start; V is fetched
    with one whole-head DMA.  Pair 1's staging is fed into pair 0's
    attention loop so PE/ACT never drain at the pair boundary.
  - exp runs on ACT (the steady-state bottleneck: ~9us/head); everything
    movable (casts, drains, normalize muls) uses nc.any so the Tile
    scheduler balances DVE/Pool.
"""

import sys
from contextlib import ExitStack

import numpy as np

sys.path.insert(0, "/opt/trn_rl_repo")

import concourse.bacc as bacc
import concourse.tile as tile
from concourse import mybir
from concourse.bass_utils import run_bass_kernel_spmd

F32 = mybir.dt.float32
BF16 = mybir.dt.bfloat16
EXP = mybir.ActivationFunctionType.Exp

B, H, T, D = 2, 16, 2048, 64
WINDOW = 512
SCALE = D ** -0.5
N_CORES = 8
HEADS_PER_CORE = (B * H) // N_CORES  # 4
TB = T // 128  # 16 query/key blocks
TH = 1024  # half-sequence tile width for qd/kd


def build_nc(t=T, heads_per_core=HEADS_PER_CORE):
    nb = t // 128

    nc = bacc.Bacc("TRN2", target_bir_lowering=False)
    q_ext = nc.declare_dram_parameter("q", [heads_per_core, t, D], F32, isOutput=False)
    k_ext = nc.declare_dram_parameter("k", [heads_per_core, t, D], F32, isOutput=False)
    v_ext = nc.declare_dram_parameter("v", [heads_per_core, t, D], F32, isOutput=False)
    id_ext = nc.declare_dram_parameter("ident", [128, 128], F32, isOutput=False)
    o_ext = nc.declare_dram_parameter("out", [heads_per_core, t, D], F32, isOutput=True)

    assert heads_per_core % 2 == 0
    n_pairs = heads_per_core // 2

    with tile.TileContext(nc) as tc, ExitStack() as ctx:
        const = ctx.enter_context(tc.tile_pool(name="const", bufs=1))
        stage = ctx.enter_context(tc.tile_pool(name="stage", bufs=6))
        vstage = ctx.enter_context(tc.tile_pool(name="vstage", bufs=2))
        qkd = ctx.enter_context(tc.tile_pool(name="qkd", bufs=2))
        vps = ctx.enter_context(tc.tile_pool(name="vps", bufs=4))
        ets = ctx.enter_context(tc.tile_pool(name="ets", bufs=13))
        outs = ctx.enter_context(tc.tile_pool(name="outs", bufs=3))
        rcp = ctx.enter_context(tc.tile_pool(name="rcp", bufs=4))
        # PSUM banks: 1 (trp) + 3*2 (sp) + 1 (shared ob/warmup) = 8
        tr_ps = ctx.enter_context(tc.tile_pool(name="tr_ps", bufs=1, space="PSUM"))
        s_ps = ctx.enter_context(tc.tile_pool(name="s_ps", bufs=3, space="PSUM"))
        ob_ps = ctx.enter_context(tc.tile_pool(name="ob_ps", bufs=1, space="PSUM"))

        # HAM warmup: the PE clock gate only opens (1.2 -> 2.4 GHz) after
        # ~3.4us of sustained matmul activity, and every real matmul here
        # runs ~2x faster warm.  Burn a dense burst of dummy matmuls on a
        # scratch PSUM bank while the first DMAs are in flight, and keep
        # feeding short dummy bursts between real matmul groups so the
        # activity monitor never re-throttles.
        dm_src = const.tile([128, 128], BF16, tag="dm_src")
        nc.vector.memset(dm_src[:], 0.0)
        dm_out = ob_ps.tile([128, 512], F32, tag="ob", name="ob_warm")

        def pe_dummy(n):
            for i in range(n):
                nc.tensor.matmul(
                    dm_out[:, 384:512], dm_src[:], dm_src[:], start=True, stop=True
                )

        pe_dummy(30)
        dm_sink = const.tile([128, 1], F32, tag="dm_sink")
        nc.vector.tensor_copy(dm_sink[:], dm_out[:, 384:385])

        # fp32 identity + bf16 copy (for Q/K transposes).
        ident_f = const.tile([128, 128], F32, tag="ident_f")
        nc.sync.dma_start(out=ident_f[:], in_=id_ext[:])
        ident_b = const.tile([128, 128], BF16, tag="ident_b")
        nc.vector.tensor_copy(ident_b[:], ident_f[:])

        # multiplicative 0/1 masks for the two boundary subtiles of E^T,
        # packed [128, 2, 128] so one strided tensor_mul masks both:
        # slot 0 keeps c >= r (causal diagonal), slot 1 keeps c < r (window).
        maskDW = const.tile([128, 2, 128], BF16, tag="maskDW")
        nc.gpsimd.memset(maskDW[:], 1.0)
        nc.gpsimd.affine_select(
            out=maskDW[:, 0, :], in_=maskDW[:, 0, :],
            compare_op=mybir.AluOpType.is_ge,
            fill=0.0, base=0, pattern=[[1, 128]], channel_multiplier=-1,
        )
        nc.gpsimd.affine_select(
            out=maskDW[:, 1, :], in_=maskDW[:, 1, :],
            compare_op=mybir.AluOpType.is_ge,
            fill=0.0, base=-1, pattern=[[-1, 128]], channel_multiplier=1,
        )

        # per-pair state
        qd_halves = {}
        kd_halves = {}
        vp = {}

        def alloc_pair(pair):
            qd_halves[pair] = [
                qkd.tile([128, TH], BF16, tag="qd0", name=f"qd0_{pair}"),
                qkd.tile([128, TH], BF16, tag="qd1", name=f"qd1_{pair}"),
            ]
            kd_halves[pair] = [
                qkd.tile([128, TH], BF16, tag="kd0", name=f"kd0_{pair}"),
                qkd.tile([128, TH], BF16, tag="kd1", name=f"kd1_{pair}"),
            ]

        def stage_unit(pair, ext, halves, u):
            # one 512-row chunk of q or k, both heads, DMA -> cast ->
            # 4 PE transposes -> drain into the d-major [128, TH] half.
            rows = slice(u * 512, (u + 1) * 512)
            st_f = stage.tile([128, 512], F32, tag="st_f")
            st3 = st_f[:].rearrange("p (b c) -> p b c", c=128)
            for eng, (hh, doff) in zip(
                (nc.sync, nc.gpsimd), ((2 * pair, 0), (2 * pair + 1, 64))
            ):
                eng.dma_start(
                    out=st3[:, :, doff : doff + 64],
                    in_=ext[hh, rows, :].rearrange("(b p) d -> p b d", p=128),
                )
            st_b = stage.tile([128, 512], BF16, tag="st_b")
            nc.vector.tensor_copy(st_b[:], st_f[:])
            trp = tr_ps.tile([128, 512], BF16, tag="trp")
            for i in range(4):
                nc.tensor.transpose(
                    trp[:, i * 128 : (i + 1) * 128],
                    st_b[:, i * 128 : (i + 1) * 128],
                    ident_b[:],
                )
            dst = halves[u // 2]
            dcol = (u % 2) * 512
            nc.vector.tensor_copy(dst[:, dcol : dcol + 512], trp[:, 0:512])

        def stage_v(h):
            vst = vstage.tile([128, 1024], F32, tag="vst")
            v3 = vst[:].rearrange("p (b d) -> p b d", d=64)
            nc.gpsimd.dma_start(
                out=v3, in_=v_ext[h].rearrange("(b p) d -> p b d", p=128)
            )
            vt = vps.tile([128, nb, 65], BF16, tag="vp", name=f"vp_{h}")
            nc.vector.tensor_copy(vt[:, :, 0:64], v3)
            nc.gpsimd.memset(vt[:, :, 64:65], 1.0)
            vp[h] = vt

        def stage_feed(pair):
            # closures that stage pair `pair`, to be interleaved into the
            # previous pair's attention loop (or run immediately).
            alloc_pair(pair)
            units = []
            units.append(lambda: stage_unit(pair, q_ext, qd_halves[pair], 0))
            units.append(lambda: stage_unit(pair, k_ext, kd_halves[pair], 0))
            units.append(lambda: stage_v(2 * pair))
            units.append(lambda: stage_v(2 * pair + 1))
            units.append(lambda: stage_unit(pair, q_ext, qd_halves[pair], 1))
            units.append(lambda: stage_unit(pair, k_ext, kd_halves[pair], 1))
            for u in (2, 3):
                units.append(lambda u=u: stage_unit(pair, q_ext, qd_halves[pair], u))
                units.append(lambda u=u: stage_unit(pair, k_ext, kd_halves[pair], u))
            return units

        def attention(pair, feed):
            # feed: dict kb -> list of closures (next pair's staging)
            hA, hB = 2 * pair, 2 * pair + 1
            rows_of = {hA: slice(0, 64), hB: slice(64, 128)}
            qdh, kdh = qd_halves[pair], kd_halves[pair]
            et = {hA: {}, hB: {}}
            sp_t = {}
            oo_t = {}
            oo3 = {}

            def emit_qk(kb):
                a = kb * 128
                span = min(640, t - a)
                for h in (hA, hB):
                    sp_t[h] = s_ps.tile([128, 1024], F32, tag="sp", name=f"sp_{h}_{kb}")
                chunks = []
                for lo in (0, TH):
                    s0, s1 = max(a, lo), min(a + span, lo + TH)
                    while s0 < s1:
                        n = min(512 - (s0 - a) % 512, s1 - s0)
                        chunks.append((s0 - a, lo // TH, s0 - lo, n))
                        s0 += n
                kd_half = kdh[a // TH]
                kcol = a % TH
                for (c, half, qc, n) in chunks:
                    for h in (hA, hB):
                        r = rows_of[h]
                        nc.tensor.matmul(
                            sp_t[h][:, c : c + n],
                            kd_half[r, kcol : kcol + 128],
                            qdh[half][r, qc : qc + n],
                            start=True,
                            stop=True,
                        )
                for h in (hA, hB):
                    e = ets.tile([128, 640], BF16, tag="et", name=f"et_{h}_{kb}")
                    et[h][kb] = e
                    nc.scalar.activation(e[:, 0:span], sp_t[h][:, 0:span], EXP, scale=SCALE)
                    if span == 640:
                        # one strided op masks both boundary triangles
                        e3 = e[:].rearrange("p (a b) -> p a b", b=128)
                        nc.vector.tensor_mul(
                            e3[:, 0:5:4, :], e3[:, 0:5:4, :], maskDW[:]
                        )
                    else:
                        nc.vector.tensor_mul(
                            e[:, 0:128], e[:, 0:128], maskDW[:, 0, :]
                        )

            ob_t = {}

            def emit_pv(qb):
                g, j = qb // 4, qb % 4
                g2, j2 = qb // 2, qb % 2
                for h in (hA, hB):
                    if j == 0:
                        oo_t[h] = outs.tile([128, 256], F32, tag="oo", name=f"oo_{h}_{g}")
                        oo3[h] = oo_t[h][:].rearrange("p (b d) -> p b d", d=64)
                    if j2 == 0 and h == hA:
                        # both heads' 2-qb accumulators share one bank-tile
                        ob_t[0] = ob_ps.tile([128, 512], F32, tag="ob", name=f"ob_{pair}_{g2}")
                    hoff = 0 if h == hA else 130
                    ob = ob_t[0][:, hoff : hoff + 130].rearrange(
                        "p (b c) -> p b c", c=65
                    )
                    kb0 = max(0, qb - 4)
                    for kb in range(kb0, qb + 1):
                        nc.tensor.matmul(
                            ob[:, j2, :],
                            et[h][kb][:, (qb - kb) * 128 : (qb - kb) * 128 + 128],
                            vp[h][:, kb, :],
                            start=(kb == kb0),
                            stop=(kb == qb),
                        )
                    if qb >= 4:
                        del et[h][qb - 4]
                    if j2 == 1:
                        # normalize 2 query blocks straight out of PSUM
                        rc = rcp.tile([128, 2], F32, tag="rc")
                        nc.vector.reciprocal(rc[:], ob[:, :, 64])
                        nc.vector.tensor_mul(
                            oo3[h][:, j - 1 : j + 1, :],
                            ob[:, :, 0:64],
                            rc[:].rearrange("p (b c) -> p b c", c=1).broadcast_to(
                                [128, 2, 64]
                            ),
                        )
                    if j == 3 or qb == nb - 1:
                        nc.sync.dma_start(
                            out=o_ext[h, g * 512 : g * 512 + 512, :].rearrange(
                                "(b p) d -> p b d", p=128
                            ),
                            in_=oo3[h][:, 0:4, :],
                        )

            for kb in range(nb + 1):
                if kb < nb:
                    emit_qk(kb)
                if kb >= 1:
                    emit_pv(kb - 1)
                for fn in feed.get(kb, ()):
                    fn()

        # pair 0: stage the first chunks inline, rest fed into its own loop
        units0 = stage_feed(0)
        for fn in units0[:6]:
            fn()
        feed0 = {0: [units0[6], units0[7]], 1: [units0[8], units0[9]]}
        # pair 1 staged during pair 0's attention, starting at kb=6
        units1 = stage_feed(1)
        feed1_in_0 = {6 + i: [units1[i]] for i in range(len(units1))}
        feed0.update(feed1_in_0)

        attention(0, feed0)
        attention(1, {})


    nc.compile()
    return nc


_NC_CACHE = {}
TRACE = False
TRACE_DIR = None
LAST_RESULT = None


def _get_nc():
    key = (T, HEADS_PER_CORE)
    if key not in _NC_CACHE:
        _NC_CACHE[key] = build_nc()
    return _NC_CACHE[key]


def kernel(q, k, v):
    q = np.ascontiguousarray(np.asarray(q, dtype=np.float32))
    k = np.ascontiguousarray(np.asarray(k, dtype=np.float32))
    v = np.ascontiguousarray(np.asarray(v, dtype=np.float32))
    assert q.shape == (B, H, T, D)

    qf = q.reshape(B * H, T, D)
    kf = k.reshape(B * H, T, D)
    vf = v.reshape(B * H, T, D)
    ident = np.eye(128, dtype=np.float32)

    in_maps = []
    for c in range(N_CORES):
        s = slice(c * HEADS_PER_CORE, (c + 1) * HEADS_PER_CORE)
        in_maps.append(
            {
                "q": np.ascontiguousarray(qf[s]),
                "k": np.ascontiguousarray(kf[s]),
                "v": np.ascontiguousarray(vf[s]),
                "ident": ident,
            }
        )

    nc = _get_nc()
    global LAST_RESULT
    res = run_bass_kernel_spmd(
        nc, in_maps, list(range(N_CORES)), trace=TRACE, tmpdir=TRACE_DIR
    )
    LAST_RESULT = res
    out = np.concatenate([res.results[c]["out"] for c in range(N_CORES)], axis=0)
    return out.reshape(B, H, T, D).astype(np.float32)
